# revision 11
# baseline (speedup 1.0000x reference)
"""Distributed Trainium2 kernel for GQA attention (nn_Attention_76845554860188).

B=1, S=2048, D=1024, NH=16, NKV=4, HD=64, causal, RoPE, 8 NeuronCores.

Sharding: tensor-parallel over heads. Core c owns q-heads {2c, 2c+1} and their
(shared, GQA) kv-head c//2. v2: the whole front of the kernel is chunk-
pipelined — xT streams in 512-position slices, Q/KV projection + RoPE for
chunk n+1 are interleaved into the attention blocks of chunk n, so the first
score matmul fires ~15us into the kernel instead of ~50us.

Scores use a full-array (128-deep) contraction that folds the q-side RoPE in:
  score = krot.q_rot = [krot; M^T krot] . [q*cos; q*sin]
so q is never explicitly rotated. kst = [krot; M^T krot] is built with two
64-contraction matmuls per chunk using host-prepared [I | M] / [M^T | I]
operators, accumulated into the projection PSUM slot after its readers drain.

The softmax denominator comes free as a ones column appended to V in the PV
matmul. exp() runs once per k-block over both heads ([128, 2, w]) on ScalarE
with the 1/sqrt(64) scale folded in; the Activation engine does nothing but
exp, the diagonal causal masks run on GpSimd, everything else element-wise on
DVE.

Output redistribution: core c owns q-128-blocks {c, 8+c}; after the last
chunk's normalize, ONE AllToAll moves all four staged chunks at once
([8, 128, 256] per core), then the o-projection runs on the received rows.
A tiny warmup AllGather at the very start absorbs the collective-stream
setup + cross-core start skew.
"""

import sys

sys.path.insert(0, "/opt/trn_rl_repo")

import numpy as np
import ml_dtypes

import concourse.bass as bass
import concourse.mybir as mybir
import concourse.tile as tile
from concourse import bacc
from concourse.bass_utils import run_bass_kernel_spmd

BF16 = mybir.dt.bfloat16
F32 = mybir.dt.float32

B, S, D = 1, 2048, 1024
NH, NKV, HD = 16, 4, 64
NC_CORES = 8
HPC = NH // NC_CORES  # q heads per core = 2
NDC = D // 128  # d chunks = 8
NSB = S // 128  # 128-wide seq blocks = 16
NCH = S // 512  # 512-wide seq chunks = 4
HALF = HD // 2  # 32

np_bf16 = ml_dtypes.bfloat16


def build_graph():
    nc = bacc.Bacc(
        "TRN2", target_bir_lowering=False, debug=False, num_devices=NC_CORES
    )

    # ---- DRAM parameters (per-core shards supplied by host) ----
    xT_e = nc.dram_tensor("xT", [D, S], BF16, kind="ExternalInput")
    wq_e = nc.dram_tensor("wq", [128, NDC, HPC * HD], BF16, kind="ExternalInput")
    wkv_e = nc.dram_tensor("wkv", [128, NDC, 2 * HD], BF16, kind="ExternalInput")
    wo_e = nc.dram_tensor("wo", [128, NDC, D], BF16, kind="ExternalInput")
    c2_e = nc.dram_tensor("c2", [128, S], BF16, kind="ExternalInput")
    s2_e = nc.dram_tensor("s2", [128, S], BF16, kind="ExternalInput")
    ka_e = nc.dram_tensor("ka", [64, 128], BF16, kind="ExternalInput")
    kb_e = nc.dram_tensor("kb", [64, 128], BF16, kind="ExternalInput")
    idm_e = nc.dram_tensor("idm", [128, 128], BF16, kind="ExternalInput")
    tri2_e = nc.dram_tensor("tri2", [128, 2 * 128], BF16, kind="ExternalInput")
    # rows [0:128] = q-block c, rows [128:256] = q-block 8+c
    out_e = nc.dram_tensor("out", [2 * 128, D], BF16, kind="ExternalOutput")

    # A2A bounce buffers: slot j = both 128-wide q-blocks destined for core j
    # ([:, :, 0:128] = q-block j, [:, :, 128:256] = q-block 8+j)
    send_d = nc.dram_tensor("a2a_send", [NC_CORES, 128, 256], BF16)
    recv_d = nc.dram_tensor("a2a_recv", [NC_CORES, 128, 256], BF16)
    # tiny warmup collective: absorbs the entry barrier + collective-stream
    # setup during the preamble so the real A2A runs at steady-state cost
    wup_s = nc.dram_tensor("wup_s", [1, 64], BF16)
    wup_r = nc.dram_tensor("wup_r", [NC_CORES, 1, 64], BF16, addr_space="Shared")

    with tile.TileContext(nc) as tc:
        _body(nc, tc, xT_e, wq_e, wkv_e, wo_e, c2_e, s2_e, ka_e, kb_e, idm_e,
              tri2_e, out_e, send_d, recv_d, wup_s, wup_r)

    nc.compile()
    return nc


def _body(nc, tc, xT_e, wq_e, wkv_e, wo_e, c2_e, s2_e, ka_e, kb_e, idm_e,
          tri2_e, out_e, send_d, recv_d, wup_s, wup_r):
    from contextlib import ExitStack

    ctx = ExitStack()
    with ctx:
        consts = ctx.enter_context(tc.tile_pool(name="consts", bufs=1))
        work = ctx.enter_context(tc.tile_pool(name="work", bufs=1))
        ptp = ctx.enter_context(tc.tile_pool(name="pt", bufs=2))
        psum = ctx.enter_context(tc.tile_pool(name="psum", bufs=1, space="PSUM"))

        # warmup collective, first in program order
        wup_sb = consts.tile([1, 64], BF16, tag="wup")
        nc.vector.memset(wup_sb[:], 0.0)
        nc.sync.dma_start(out=wup_s.ap(), in_=wup_sb[:])
        nc.gpsimd.collective_compute(
            "AllGather",
            mybir.AluOpType.bypass,
            replica_groups=[list(range(NC_CORES))],
            ins=[wup_s.ap().opt()],
            outs=[wup_r.ap().opt()],
        )

        # ---- parameter loads: small weights first, then xT s-chunk-major ----
        wq_sb = consts.tile([128, NDC, HPC * HD], BF16, tag="wq")
        nc.scalar.dma_start(out=wq_sb[:], in_=wq_e.ap())
        wkv_sb = consts.tile([128, NDC, 2 * HD], BF16, tag="wkv")
        nc.scalar.dma_start(out=wkv_sb[:], in_=wkv_e.ap())
        ka_sb = consts.tile([64, 128], BF16, tag="ka")
        nc.sync.dma_start(out=ka_sb[:], in_=ka_e[:, :])
        kb_sb = consts.tile([64, 128], BF16, tag="kb")
        nc.sync.dma_start(out=kb_sb[:], in_=kb_e[:, :])
        idm_sb = consts.tile([128, 128], BF16, tag="idm")
        nc.sync.dma_start(out=idm_sb[:], in_=idm_e[:, :])
        tri2_sb = consts.tile([128, 2, 128], BF16, tag="tri2")
        nc.sync.dma_start(
            out=tri2_sb[:], in_=tri2_e.ap().rearrange("p (h n) -> p h n", h=2)
        )
        c2_sb = consts.tile([128, S], BF16, tag="c2")
        nc.sync.dma_start(out=c2_sb[:], in_=c2_e[:, :])
        s2_sb = consts.tile([128, S], BF16, tag="s2")
        nc.scalar.dma_start(out=s2_sb[:], in_=s2_e[:, :])

        # xT arrives one 512-position chunk at a time (all 8 d-blocks), so
        # chunk-0 projections can start ~2us after the queues open
        xT_sb = consts.tile([128, NDC, S], BF16, tag="xT")
        qeng = [nc.sync, nc.scalar, nc.gpsimd]
        for n in range(NCH):
            qeng[n % 3].dma_start(
                out=xT_sb[:, :, 512 * n : 512 * (n + 1)],
                in_=xT_e.ap()[:, 512 * n : 512 * (n + 1)].rearrange(
                    "(i p) n -> p i n", p=128
                ),
            )
        # Wo prefetch on the gpsimd queue: early so the A2A rings stay quiet
        wo_sb = consts.tile([128, NDC, D], BF16, tag="wo")
        for i in range(NDC):
            nc.gpsimd.dma_start(out=wo_sb[:, i, :], in_=wo_e[:, i, :])

        # ---- persistent SBUF state ----
        kst_sb = work.tile([128, S], BF16, tag="kst")
        qcs = work.tile([128, HPC, S], BF16, tag="qcs")
        vext_sb = work.tile([128, NSB, HD + 1], BF16, tag="vext")
        nc.vector.memset(vext_sb[:, :, HD : HD + 1], 1.0)

        scale = 1.0 / np.sqrt(HD)

        def proj_rope_a(n):
            """Chunk n: Q/KV projection matmuls + K-side rope element-wise."""
            sl = slice(512 * n, 512 * (n + 1))
            pj = psum.tile([128, 2, 512], F32, tag="pj", bufs=1, name=f"pj{n}")
            for i in range(NDC):
                nc.tensor.matmul(
                    pj[:, 1, :], lhsT=wkv_sb[:, i, :], rhs=xT_sb[:, i, sl],
                    start=(i == 0), stop=(i == NDC - 1),
                )
            for i in range(NDC):
                nc.tensor.matmul(
                    pj[:, 0, :], lhsT=wq_sb[:, i, :], rhs=xT_sb[:, i, sl],
                    start=(i == 0), stop=(i == NDC - 1),
                )
            # K-side rope factors + V copy off the kv PSUM slot
            kc_t = work.tile([64, 512], BF16, tag="kc", bufs=2, name=f"kc{n}")
            nc.vector.tensor_tensor(
                out=kc_t[:], in0=pj[0:64, 1, :], in1=c2_sb[0:64, sl],
                op=mybir.AluOpType.mult,
            )
            ks_t = work.tile([64, 512], BF16, tag="ks", bufs=2, name=f"ks{n}")
            nc.vector.tensor_tensor(
                out=ks_t[:], in0=pj[0:64, 1, :], in1=s2_sb[0:64, sl],
                op=mybir.AluOpType.mult,
            )
            v_t = work.tile([64, 512], BF16, tag="vs", bufs=2, name=f"vs{n}")
            nc.vector.tensor_copy(out=v_t[:], in_=pj[64:128, 1, :])
            return pj, kc_t, ks_t, v_t

        def proj_rope_b(n, parts):
            """Chunk n: kst build (into the drained kv slot), q-side rope,
            V transpose (ring slot after pj), vext staging."""
            sl = slice(512 * n, 512 * (n + 1))
            pj, kc_t, ks_t, v_t = parts
            # kst = [I|M].kc + [M^T|I].ks, accumulated over the kv slot (WAR)
            nc.tensor.matmul(
                pj[:, 1, :], lhsT=ka_sb[:], rhs=kc_t[:], start=True, stop=False
            )
            nc.tensor.matmul(
                pj[:, 1, :], lhsT=kb_sb[:], rhs=ks_t[:], start=False, stop=True
            )
            nc.vector.tensor_copy(out=kst_sb[:, sl], in_=pj[:, 1, :])
            # q-side rope halves: qcs[h] = [q_h * cos; q_h * sin]
            for h in range(HPC):
                nc.vector.tensor_tensor(
                    out=qcs[0:64, h, sl], in0=pj[64 * h : 64 * (h + 1), 0, :],
                    in1=c2_sb[64 * h : 64 * (h + 1), sl],
                    op=mybir.AluOpType.mult,
                )
                nc.vector.tensor_tensor(
                    out=qcs[64:128, h, sl], in0=pj[64 * h : 64 * (h + 1), 0, :],
                    in1=s2_sb[64 * h : 64 * (h + 1), sl],
                    op=mybir.AluOpType.mult,
                )
            # V transpose into the next pj ring slot (bf16), then to vext
            vt = psum.tile([128, 4, HD], BF16, tag="pj", bufs=1, name=f"vt{n}")
            for b in range(4):
                nc.tensor.transpose(
                    vt[:, b, :], v_t[:, 128 * b : 128 * (b + 1)],
                    idm_sb[0:64, 0:64],
                )
            nc.vector.tensor_copy(
                out=vext_sb[:, 4 * n : 4 * (n + 1), 0:HD], in_=vt[:]
            )

        def pv(kc, b, nb, pt, ot_t):
            cbase = 512 * kc
            q0 = max(cbase, 128 * b)
            w = cbase + 512 - q0
            for h in range(HPC):
                nc.tensor.matmul(
                    ot_t[:, h, q0 - cbase : 512],
                    lhsT=vext_sb[:, b, :],
                    rhs=pt[:, h, 0:w],
                    start=(b == 0),
                    stop=(b == nb - 1),
                )

        def attention(kc):
            """Causal flash attention for q-chunk kc; chunk kc+1's projection
            work is injected after block 1 so PE fills the exp-wait gaps."""
            cbase = 512 * kc
            nb = 4 * kc + 4
            ot_t = psum.tile([HD + 1, HPC, 512], F32, tag="ot", bufs=1,
                             name=f"ot{kc}")
            prev = None
            for b in range(nb):
                if b == min(2, nb - 1) and kc + 1 < NCH:
                    parts = proj_rope_a(kc + 1)
                if b == min(4, nb - 1) and kc + 1 < NCH:
                    proj_rope_b(kc + 1, parts)
                q0 = max(cbase, 128 * b)
                w = cbase + 512 - q0
                st_ps = psum.tile([128, HPC, 512], F32, tag="st", bufs=2)
                for h in range(HPC):
                    nc.tensor.matmul(
                        st_ps[:, h, 0:w],
                        lhsT=kst_sb[:, 128 * b : 128 * (b + 1)],
                        rhs=qcs[:, h, q0 : q0 + w],
                        start=True,
                        stop=True,
                    )
                pt = ptp.tile([128, HPC, 512], BF16, tag="pt", bufs=3)
                nc.scalar.activation(
                    out=pt[:, :, 0:w],
                    in_=st_ps[:, :, 0:w],
                    func=mybir.ActivationFunctionType.Exp,
                    scale=scale,
                )
                if 128 * b >= cbase:
                    # diagonal block: mask the leading 128x128 (sq < sk -> 0)
                    nc.gpsimd.tensor_tensor(
                        out=pt[:, :, 0:128],
                        in0=pt[:, :, 0:128],
                        in1=tri2_sb[:],
                        op=mybir.AluOpType.mult,
                    )
                if prev is not None:
                    pv(kc, b - 1, nb, prev, ot_t)
                prev = pt
            pv(kc, nb - 1, nb, prev, ot_t)

            # normalize: row HD of ot is the softmax denominator
            stg = work.tile([128, 512], BF16, tag="stg", bufs=2, name=f"stg{kc}")
            for h in range(HPC):
                den_sb = work.tile([1, 512], F32, tag="den", bufs=4)
                nc.vector.tensor_copy(out=den_sb[:], in_=ot_t[HD : HD + 1, h, :])
                rec_sb = work.tile([1, 512], F32, tag="rec", bufs=4)
                nc.vector.reciprocal_approx_fast(out=rec_sb[:], in_=den_sb[:])
                bcr_sb = work.tile([HD, 512], F32, tag="bcr", bufs=2)
                nc.gpsimd.partition_broadcast(bcr_sb[:], rec_sb[:])
                nc.vector.tensor_tensor(
                    out=stg[64 * h : 64 * (h + 1), :],
                    in0=ot_t[0:HD, h, :],
                    in1=bcr_sb[:],
                    op=mybir.AluOpType.mult,
                )
            # stage chunk kc = q-blocks {4kc..4kc+3}: slot qb%8, col-half qb//8
            eng = nc.sync if kc % 2 == 0 else nc.scalar
            eng.dma_start(
                out=send_d.ap()[
                    4 * (kc % 2) : 4 * (kc % 2) + 4,
                    :,
                    128 * (kc // 2) : 128 * (kc // 2) + 128,
                ].rearrange("j p n -> p j n"),
                in_=stg[:],
            )

        # ---- main pipeline ----
        parts0 = proj_rope_a(0)
        proj_rope_b(0, parts0)
        for kc in range(NCH):
            attention(kc)

        # ---- one AllToAll for all four chunks, then the o-projection ----
        nc.gpsimd.collective_compute(
            "AllToAll",
            mybir.AluOpType.bypass,
            replica_groups=[list(range(NC_CORES))],
            ins=[send_d.ap().opt()],
            outs=[recv_d.ap().opt()],
        )
        at_sb = work.tile([128, NC_CORES, 256], BF16, tag="at")
        nc.sync.dma_start(
            out=at_sb[:, 0:4, :],
            in_=recv_d.ap()[0:4].rearrange("s p n -> p s n"),
        )
        nc.scalar.dma_start(
            out=at_sb[:, 4:8, :],
            in_=recv_d.ap()[4:8].rearrange("s p n -> p s n"),
        )
        for half in range(2):
            ou_sb = work.tile([128, D], BF16, tag="ou", bufs=2, name=f"ou{half}")
            for dn in range(2):
                op_ps = psum.tile([128, 512], F32, tag="st", bufs=2,
                                  name=f"op{half}_{dn}")
                for j in range(NC_CORES):
                    nc.tensor.matmul(
                        op_ps[:, :],
                        lhsT=at_sb[:, j, 128 * half : 128 * (half + 1)],
                        rhs=wo_sb[:, j, 512 * dn : 512 * (dn + 1)],
                        start=(j == 0),
                        stop=(j == NC_CORES - 1),
                    )
                nc.vector.tensor_copy(
                    out=ou_sb[:, 512 * dn : 512 * (dn + 1)], in_=op_ps[:]
                )
            nc.scalar.dma_start(
                out=out_e.ap()[128 * half : 128 * (half + 1), :], in_=ou_sb[:]
            )


# ---------------- host side ----------------

_CACHE = {}


def _prep_consts():
    # M: signed half-swap for one 64-wide head (rotate-half convention)
    M = np.zeros((64, 64), np.float32)
    for j in range(HALF):
        M[j, HALF + j] = -1.0
        M[HALF + j, j] = 1.0
    I = np.eye(64, dtype=np.float32)
    # kst = [[I],[M^T]] @ kc + [[M],[I]] @ ks -> lhsT operators [64, 128]
    ka = np.concatenate([I, M], axis=1).astype(np_bf16)
    kb = np.concatenate([M.T, I], axis=1).astype(np_bf16)
    idm = np.eye(128, dtype=np_bf16)
    # tri[p, j] = 1 if j >= p (valid: sq >= sk within diagonal block)
    tri = (np.arange(128)[None, :] >= np.arange(128)[:, None]).astype(np_bf16)
    tri2 = np.concatenate([tri, tri], axis=1)  # [128, 2*128], per-head copy
    return ka, kb, idm, tri2


def kernel(x, rope_cos, rope_sin, Wq, Wk, Wv, Wo):
    if "nc" not in _CACHE:
        _CACHE["nc"] = build_graph()
    nc = _CACHE["nc"]

    x2 = np.asarray(x, np.float32).reshape(S, D)
    xT = np.ascontiguousarray(x2.T).astype(np_bf16)
    cosT = np.asarray(rope_cos, np.float32).T  # [32, S]
    sinT = np.asarray(rope_sin, np.float32).T
    c2 = np.tile(cosT, (4, 1)).astype(np_bf16)  # [128, S]
    s2 = np.tile(sinT, (4, 1)).astype(np_bf16)
    ka, kb, idm, tri2 = _prep_consts()

    Wq = np.asarray(Wq, np.float32)
    Wk = np.asarray(Wk, np.float32)
    Wv = np.asarray(Wv, np.float32)
    Wo = np.asarray(Wo, np.float32)

    def chunked(w):  # [1024, X] -> [128, 8, X] (partition-major d-chunks)
        return np.ascontiguousarray(
            w.reshape(NDC, 128, -1).transpose(1, 0, 2)
        ).astype(np_bf16)

    wo_b = chunked(Wo)
    in_maps = []
    for c in range(NC_CORES):
        kv = c // 2
        wq_c = chunked(Wq[:, HPC * HD * c : HPC * HD * (c + 1)])
        wkv_c = chunked(
            np.concatenate(
                [Wk[:, HD * kv : HD * (kv + 1)], Wv[:, HD * kv : HD * (kv + 1)]],
                axis=1,
            )
        )
        in_maps.append(
            {
                "xT": xT,
                "wq": wq_c,
                "wkv": wkv_c,
                "wo": wo_b,
                "c2": c2,
                "s2": s2,
                "ka": ka,
                "kb": kb,
                "idm": idm,
                "tri2": tri2,
            }
        )

    res = run_bass_kernel_spmd(nc, in_maps, core_ids=list(range(NC_CORES)))
    out = np.zeros((S, D), np.float32)
    for c in range(NC_CORES):
        blk = np.asarray(res.results[c]["out"], np.float32)
        out[128 * c : 128 * (c + 1)] = blk[0:128]
        out[128 * (8 + c) : 128 * (9 + c)] = blk[128:256]
    return out.reshape(B, S, D)


# revision 24
# speedup vs baseline: 1.2441x; 1.2441x over previous
"""Distributed Trainium2 kernel for GQA attention (nn_Attention_76845554860188).

B=1, S=2048, D=1024, NH=16, NKV=4, HD=64, causal, RoPE, 8 NeuronCores.

Sharding: tensor-parallel over heads. Core c owns q-heads {2c, 2c+1} and their
(shared, GQA) kv-head c//2. v2: the whole front of the kernel is chunk-
pipelined — xT streams in 512-position slices, Q/KV projection + RoPE for
chunk n+1 are interleaved into the attention blocks of chunk n, so the first
score matmul fires ~15us into the kernel instead of ~50us.

Scores use a full-array (128-deep) contraction that folds the q-side RoPE in:
  score = krot.q_rot = [krot; M^T krot] . [q*cos; q*sin]
so q is never explicitly rotated. kst = [krot; M^T krot] is built with two
64-contraction matmuls per chunk using host-prepared [I | M] / [M^T | I]
operators, accumulated into the projection PSUM slot after its readers drain.

The softmax denominator comes free as a ones column appended to V in the PV
matmul. exp() runs once per k-block over both heads ([128, 2, w]) on ScalarE
with the 1/sqrt(64) scale folded in; the Activation engine does nothing but
exp, the diagonal causal masks run on GpSimd, everything else element-wise on
DVE.

Output redistribution: core c owns q-128-blocks {c, 8+c}; after the last
chunk's normalize, ONE AllToAll moves all four staged chunks at once
([8, 128, 256] per core), then the o-projection runs on the received rows.
A tiny warmup AllGather at the very start absorbs the collective-stream
setup + cross-core start skew.
"""

import sys

sys.path.insert(0, "/opt/trn_rl_repo")

import numpy as np
import ml_dtypes

import concourse.bass as bass
import concourse.mybir as mybir
import concourse.tile as tile
from concourse import bacc
from concourse.bass_utils import run_bass_kernel_spmd

BF16 = mybir.dt.bfloat16
F32 = mybir.dt.float32

B, S, D = 1, 2048, 1024
NH, NKV, HD = 16, 4, 64
NC_CORES = 8
HPC = NH // NC_CORES  # q heads per core = 2
NDC = D // 128  # d chunks = 8
NSB = S // 128  # 128-wide seq blocks = 16
NCH = S // 512  # 512-wide seq chunks = 4
HALF = HD // 2  # 32

np_bf16 = ml_dtypes.bfloat16


def build_graph():
    nc = bacc.Bacc(
        "TRN2", target_bir_lowering=False, debug=False, num_devices=NC_CORES
    )

    # ---- DRAM parameters (per-core shards supplied by host) ----
    xT_e = nc.dram_tensor("xT", [D, S], BF16, kind="ExternalInput")
    wq_e = nc.dram_tensor("wq", [128, NDC, HPC * HD], BF16, kind="ExternalInput")
    wkv_e = nc.dram_tensor("wkv", [128, NDC, 2 * HD], BF16, kind="ExternalInput")
    wo_e = nc.dram_tensor("wo", [128, NDC, D], BF16, kind="ExternalInput")
    c2_e = nc.dram_tensor("c2", [128, S], BF16, kind="ExternalInput")
    s2_e = nc.dram_tensor("s2", [128, S], BF16, kind="ExternalInput")
    # packed small constants: idm [0:128] | tri2 [128:384] | ka [384:512] |
    # kb [512:640] (ka/kb in rows 0:64)
    cst_e = nc.dram_tensor("cst", [128, 640], BF16, kind="ExternalInput")
    # rows [0:128] = q-block c, rows [128:256] = q-block 8+c
    out_e = nc.dram_tensor("out", [2 * 128, D], BF16, kind="ExternalOutput")

    # A2A bounce buffers: slot j = both 128-wide q-blocks destined for core j
    # ([:, :, 0:128] = q-block j, [:, :, 128:256] = q-block 8+j)
    send_d = nc.dram_tensor("a2a_send", [NC_CORES, 128, 256], BF16)
    recv_d = nc.dram_tensor("a2a_recv", [NC_CORES, 128, 256], BF16)
    # tiny warmup collective: absorbs the entry barrier + collective-stream
    # setup during the preamble so the real A2A runs at steady-state cost
    wup_s = nc.dram_tensor("wup_s", [1, 64], BF16)
    wup_r = nc.dram_tensor("wup_r", [NC_CORES, 1, 64], BF16, addr_space="Shared")

    with tile.TileContext(nc) as tc:
        _body(nc, tc, xT_e, wq_e, wkv_e, wo_e, c2_e, s2_e, cst_e,
              out_e, send_d, recv_d, wup_s, wup_r)

    nc.compile()
    return nc


def _body(nc, tc, xT_e, wq_e, wkv_e, wo_e, c2_e, s2_e, cst_e,
          out_e, send_d, recv_d, wup_s, wup_r):
    from contextlib import ExitStack

    ctx = ExitStack()
    with ctx:
        consts = ctx.enter_context(tc.tile_pool(name="consts", bufs=1))
        work = ctx.enter_context(tc.tile_pool(name="work", bufs=1))
        ptp = ctx.enter_context(tc.tile_pool(name="pt", bufs=2))
        psum = ctx.enter_context(tc.tile_pool(name="psum", bufs=1, space="PSUM"))

        # warmup collective, first in program order (payload is garbage DRAM;
        # only the barrier/stream-setup side effect matters)
        nc.gpsimd.collective_compute(
            "AllGather",
            mybir.AluOpType.bypass,
            replica_groups=[list(range(NC_CORES))],
            ins=[wup_s.ap().opt()],
            outs=[wup_r.ap().opt()],
        )

        # ---- parameter loads, in need-order: xT chunk 0 + projection
        # weights first, rope tables next, Wo (needed last) at the back ----
        wq_sb = consts.tile([128, NDC, HPC * HD], BF16, tag="wq")
        nc.scalar.dma_start(out=wq_sb[:], in_=wq_e.ap())
        wkv_sb = consts.tile([128, NDC, 2 * HD], BF16, tag="wkv")
        nc.scalar.dma_start(out=wkv_sb[:], in_=wkv_e.ap())
        cst_sb = consts.tile([128, 640], BF16, tag="cst")
        nc.sync.dma_start(out=cst_sb[:], in_=cst_e.ap())
        idm_sb = cst_sb[:, 0:128]
        tri2_sb = cst_sb[:, 128:384].rearrange("p (h n) -> p h n", h=2)
        ka_sb = cst_sb[0:64, 384:512]
        kb_sb = cst_sb[0:64, 512:640]

        # xT arrives one 512-position chunk at a time (all 8 d-blocks), so
        # chunk-0 projections can start ~2us after the queues open
        xT_sb = consts.tile([128, NDC, S], BF16, tag="xT")
        qeng = [nc.sync, nc.scalar, nc.gpsimd, nc.sync]
        for n in range(NCH):
            qeng[n].dma_start(
                out=xT_sb[:, :, 512 * n : 512 * (n + 1)],
                in_=xT_e.ap()[:, 512 * n : 512 * (n + 1)].rearrange(
                    "(i p) n -> p i n", p=128
                ),
            )
        c2_sb = consts.tile([128, S], BF16, tag="c2")
        nc.scalar.dma_start(out=c2_sb[:], in_=c2_e[:, :])
        s2_sb = consts.tile([128, S], BF16, tag="s2")
        nc.scalar.dma_start(out=s2_sb[:], in_=s2_e[:, :])
        # Wo prefetch on the gpsimd queue: early so the A2A rings stay quiet
        wo_sb = consts.tile([128, NDC, D], BF16, tag="wo")
        for i in range(NDC):
            nc.gpsimd.dma_start(out=wo_sb[:, i, :], in_=wo_e[:, i, :])

        # ---- persistent SBUF state ----
        kst_sb = work.tile([128, S], BF16, tag="kst")
        qcs = work.tile([128, HPC, S], BF16, tag="qcs")
        vext_sb = work.tile([128, NSB, HD + 1], BF16, tag="vext")
        nc.vector.memset(vext_sb[:, :, HD : HD + 1], 1.0)

        scale = 1.0 / np.sqrt(HD)

        def proj_rope_a(n):
            """Chunk n: Q/KV projection matmuls + K-side rope element-wise."""
            sl = slice(512 * n, 512 * (n + 1))
            pj = psum.tile([128, 2, 512], F32, tag="pj", bufs=1, name=f"pj{n}")
            for i in range(NDC):
                nc.tensor.matmul(
                    pj[:, 1, :], lhsT=wkv_sb[:, i, :], rhs=xT_sb[:, i, sl],
                    start=(i == 0), stop=(i == NDC - 1),
                )
            for i in range(NDC):
                nc.tensor.matmul(
                    pj[:, 0, :], lhsT=wq_sb[:, i, :], rhs=xT_sb[:, i, sl],
                    start=(i == 0), stop=(i == NDC - 1),
                )
            # K-side rope factors + V copy off the kv PSUM slot
            kc_t = work.tile([64, 512], BF16, tag="kc", bufs=2, name=f"kc{n}")
            nc.vector.tensor_tensor(
                out=kc_t[:], in0=pj[0:64, 1, :], in1=c2_sb[0:64, sl],
                op=mybir.AluOpType.mult,
            )
            ks_t = work.tile([64, 512], BF16, tag="ks", bufs=2, name=f"ks{n}")
            nc.vector.tensor_tensor(
                out=ks_t[:], in0=pj[0:64, 1, :], in1=s2_sb[0:64, sl],
                op=mybir.AluOpType.mult,
            )
            v_t = work.tile([64, 512], BF16, tag="vs", bufs=2, name=f"vs{n}")
            nc.vector.tensor_copy(out=v_t[:], in_=pj[64:128, 1, :])
            return pj, kc_t, ks_t, v_t

        def proj_rope_b(n, parts):
            """Chunk n: kst build (into the drained kv slot), q-side rope,
            V transpose (ring slot after pj), vext staging."""
            sl = slice(512 * n, 512 * (n + 1))
            pj, kc_t, ks_t, v_t = parts
            # kst = [I|M].kc + [M^T|I].ks, accumulated over the kv slot (WAR)
            nc.tensor.matmul(
                pj[:, 1, :], lhsT=ka_sb[:], rhs=kc_t[:], start=True, stop=False
            )
            nc.tensor.matmul(
                pj[:, 1, :], lhsT=kb_sb[:], rhs=ks_t[:], start=False, stop=True
            )
            nc.vector.tensor_copy(out=kst_sb[:, sl], in_=pj[:, 1, :])
            # q-side rope halves: qcs[h] = [q_h * cos; q_h * sin]
            for h in range(HPC):
                nc.vector.tensor_tensor(
                    out=qcs[0:64, h, sl], in0=pj[64 * h : 64 * (h + 1), 0, :],
                    in1=c2_sb[64 * h : 64 * (h + 1), sl],
                    op=mybir.AluOpType.mult,
                )
                nc.vector.tensor_tensor(
                    out=qcs[64:128, h, sl], in0=pj[64 * h : 64 * (h + 1), 0, :],
                    in1=s2_sb[64 * h : 64 * (h + 1), sl],
                    op=mybir.AluOpType.mult,
                )
            # V transpose into the next pj ring slot (bf16), then to vext
            vt = psum.tile([128, 4, HD], BF16, tag="pj", bufs=1, name=f"vt{n}")
            for b in range(4):
                nc.tensor.transpose(
                    vt[:, b, :], v_t[:, 128 * b : 128 * (b + 1)],
                    idm_sb[0:64, 0:64],
                )
            nc.vector.tensor_copy(
                out=vext_sb[:, 4 * n : 4 * (n + 1), 0:HD], in_=vt[:]
            )

        def pv(kc, b, nb, pt, ot_t):
            cbase = 512 * kc
            q0 = max(cbase, 128 * b)
            w = cbase + 512 - q0
            for h in range(HPC):
                nc.tensor.matmul(
                    ot_t[:, h, q0 - cbase : 512],
                    lhsT=vext_sb[:, b, :],
                    rhs=pt[:, h, 0:w],
                    start=(b == 0),
                    stop=(b == nb - 1),
                )

        def attention(kc):
            """Causal flash attention for q-chunk kc; chunk kc+1's projection
            work is injected after block 1 so PE fills the exp-wait gaps."""
            cbase = 512 * kc
            nb = 4 * kc + 4
            ot_t = psum.tile([HD + 1, HPC, 512], F32, tag="ot", bufs=1,
                             name=f"ot{kc}")
            prev = None
            for b in range(nb):
                if b == min(2, nb - 1) and kc + 1 < NCH:
                    parts = proj_rope_a(kc + 1)
                if b == min(4, nb - 1) and kc + 1 < NCH:
                    proj_rope_b(kc + 1, parts)
                q0 = max(cbase, 128 * b)
                w = cbase + 512 - q0
                st_ps = psum.tile([128, HPC, 512], F32, tag="st", bufs=2)
                for h in range(HPC):
                    nc.tensor.matmul(
                        st_ps[:, h, 0:w],
                        lhsT=kst_sb[:, 128 * b : 128 * (b + 1)],
                        rhs=qcs[:, h, q0 : q0 + w],
                        start=True,
                        stop=True,
                    )
                pt = ptp.tile([128, HPC, 512], BF16, tag="pt", bufs=3)
                nc.scalar.activation(
                    out=pt[:, :, 0:w],
                    in_=st_ps[:, :, 0:w],
                    func=mybir.ActivationFunctionType.Exp,
                    scale=scale,
                )
                if 128 * b >= cbase:
                    # diagonal block: mask the leading 128x128 (sq < sk -> 0)
                    # on DVE — gpsimd must stay PartitionBroadcast-only, a
                    # mixed op set forces ~5us Q7 library reloads per switch
                    nc.vector.tensor_tensor(
                        out=pt[:, :, 0:128],
                        in0=pt[:, :, 0:128],
                        in1=tri2_sb,
                        op=mybir.AluOpType.mult,
                    )
                if prev is not None:
                    pv(kc, b - 1, nb, prev, ot_t)
                prev = pt
            pv(kc, nb - 1, nb, prev, ot_t)

            # normalize: row HD of ot is the softmax denominator
            stg = work.tile([128, 512], BF16, tag="stg", bufs=2, name=f"stg{kc}")
            for h in range(HPC):
                den_sb = work.tile([1, 512], F32, tag="den", bufs=4)
                nc.vector.tensor_copy(out=den_sb[:], in_=ot_t[HD : HD + 1, h, :])
                rec_sb = work.tile([1, 512], F32, tag="rec", bufs=4)
                nc.vector.reciprocal_approx_fast(out=rec_sb[:], in_=den_sb[:])
                bcr_sb = work.tile([HD, 512], F32, tag="bcr", bufs=2)
                nc.gpsimd.partition_broadcast(bcr_sb[:], rec_sb[:])
                nc.vector.tensor_tensor(
                    out=stg[64 * h : 64 * (h + 1), :],
                    in0=ot_t[0:HD, h, :],
                    in1=bcr_sb[:],
                    op=mybir.AluOpType.mult,
                )
            # stage chunk kc = q-blocks {4kc..4kc+3}: slot qb%8, col-half qb//8
            eng = nc.sync if kc % 2 == 0 else nc.scalar
            eng.dma_start(
                out=send_d.ap()[
                    4 * (kc % 2) : 4 * (kc % 2) + 4,
                    :,
                    128 * (kc // 2) : 128 * (kc // 2) + 128,
                ].rearrange("j p n -> p j n"),
                in_=stg[:],
            )

        # ---- main pipeline ----
        parts0 = proj_rope_a(0)
        proj_rope_b(0, parts0)
        for kc in range(NCH):
            attention(kc)

        # ---- one AllToAll for all four chunks, then the o-projection ----
        nc.gpsimd.collective_compute(
            "AllToAll",
            mybir.AluOpType.bypass,
            replica_groups=[list(range(NC_CORES))],
            ins=[send_d.ap().opt()],
            outs=[recv_d.ap().opt()],
        )
        at_sb = work.tile([128, NC_CORES, 256], BF16, tag="at")
        nc.sync.dma_start(
            out=at_sb[:, 0:4, :],
            in_=recv_d.ap()[0:4].rearrange("s p n -> p s n"),
        )
        nc.scalar.dma_start(
            out=at_sb[:, 4:8, :],
            in_=recv_d.ap()[4:8].rearrange("s p n -> p s n"),
        )
        for half in range(2):
            ou_sb = work.tile([128, D], BF16, tag="ou", bufs=2, name=f"ou{half}")
            for dn in range(2):
                op_ps = psum.tile([128, 512], F32, tag="st", bufs=2,
                                  name=f"op{half}_{dn}")
                for j in range(NC_CORES):
                    nc.tensor.matmul(
                        op_ps[:, :],
                        lhsT=at_sb[:, j, 128 * half : 128 * (half + 1)],
                        rhs=wo_sb[:, j, 512 * dn : 512 * (dn + 1)],
                        start=(j == 0),
                        stop=(j == NC_CORES - 1),
                    )
                nc.vector.tensor_copy(
                    out=ou_sb[:, 512 * dn : 512 * (dn + 1)], in_=op_ps[:]
                )
            nc.scalar.dma_start(
                out=out_e.ap()[128 * half : 128 * (half + 1), :], in_=ou_sb[:]
            )


# ---------------- host side ----------------

_CACHE = {}


def _prep_consts():
    # M: signed half-swap for one 64-wide head (rotate-half convention)
    M = np.zeros((64, 64), np.float32)
    for j in range(HALF):
        M[j, HALF + j] = -1.0
        M[HALF + j, j] = 1.0
    I = np.eye(64, dtype=np.float32)
    # kst = [[I],[M^T]] @ kc + [[M],[I]] @ ks -> lhsT operators [64, 128]
    ka = np.concatenate([I, M], axis=1).astype(np.float32)
    kb = np.concatenate([M.T, I], axis=1).astype(np.float32)
    idm = np.eye(128, dtype=np.float32)
    # tri[p, j] = 1 if j >= p (valid: sq >= sk within diagonal block)
    tri = (np.arange(128)[None, :] >= np.arange(128)[:, None]).astype(np.float32)
    tri2 = np.concatenate([tri, tri], axis=1)  # [128, 2*128], per-head copy
    # pack: idm | tri2 | ka(pad) | kb(pad) -> [128, 640]
    kap = np.zeros((128, 128), np.float32)
    kap[0:64] = ka
    kbp = np.zeros((128, 128), np.float32)
    kbp[0:64] = kb
    cst = np.concatenate([idm, tri2, kap, kbp], axis=1).astype(np_bf16)
    return cst


def kernel(x, rope_cos, rope_sin, Wq, Wk, Wv, Wo):
    if "nc" not in _CACHE:
        _CACHE["nc"] = build_graph()
    nc = _CACHE["nc"]

    x2 = np.asarray(x, np.float32).reshape(S, D)
    xT = np.ascontiguousarray(x2.T).astype(np_bf16)
    cosT = np.asarray(rope_cos, np.float32).T  # [32, S]
    sinT = np.asarray(rope_sin, np.float32).T
    c2 = np.tile(cosT, (4, 1)).astype(np_bf16)  # [128, S]
    s2 = np.tile(sinT, (4, 1)).astype(np_bf16)
    cst = _prep_consts()

    Wq = np.asarray(Wq, np.float32)
    Wk = np.asarray(Wk, np.float32)
    Wv = np.asarray(Wv, np.float32)
    Wo = np.asarray(Wo, np.float32)

    def chunked(w):  # [1024, X] -> [128, 8, X] (partition-major d-chunks)
        return np.ascontiguousarray(
            w.reshape(NDC, 128, -1).transpose(1, 0, 2)
        ).astype(np_bf16)

    wo_b = chunked(Wo)
    in_maps = []
    for c in range(NC_CORES):
        kv = c // 2
        wq_c = chunked(Wq[:, HPC * HD * c : HPC * HD * (c + 1)])
        wkv_c = chunked(
            np.concatenate(
                [Wk[:, HD * kv : HD * (kv + 1)], Wv[:, HD * kv : HD * (kv + 1)]],
                axis=1,
            )
        )
        in_maps.append(
            {
                "xT": xT,
                "wq": wq_c,
                "wkv": wkv_c,
                "wo": wo_b,
                "c2": c2,
                "s2": s2,
                "cst": cst,
            }
        )

    res = run_bass_kernel_spmd(nc, in_maps, core_ids=list(range(NC_CORES)))
    out = np.zeros((S, D), np.float32)
    for c in range(NC_CORES):
        blk = np.asarray(res.results[c]["out"], np.float32)
        out[128 * c : 128 * (c + 1)] = blk[0:128]
        out[128 * (8 + c) : 128 * (9 + c)] = blk[128:256]
    return out.reshape(B, S, D)
